# revision 34
# baseline (speedup 1.0000x reference)
"""Multi-head self-attention (b=4, s=2048, d_model=1024, h=16, causal) on 8 trn2 cores.

Sharding: core c = (batch b = c//2, head-group g = c%2): 8 heads of one batch
per core, full QKV + causal attention + partial W_o projection on device; host
pre-transposes x/W slices and sums the two partial y's per batch (the W_o
all-reduce done at unshard time).

All matmul operands are fp16 (full PE stream rate) with fp32 PSUM
accumulation. x and the weights arrive as host-prearranged [128, ...] DRAM
tensors -> one or four DMAs each, ordered so the first projection matmul only
waits on chunk-0 x + pair-0 W_q/W_k.

The exp stream is split between the scalar engine (ACT table exp, with the
softmax scale/bias folded into the activation's free affine) and a custom
8-stage DVE op for ~1/3 of the full key-tiles: Q is pre-scaled by
0.125/sqrt(2048) in its PSUM->SBUF copy so exp(z-3) factors as
sq^5(sq(s' + C0) + 0.5) = ((z+29)^2/2048 + 0.5)^32 -- a second-order
(1+u)^n expansion, ~0.2% end-to-end error, registered at import via the
dve_ops extension list. Other work is interleaved INTO the attention
stretch: after pair p of chunk j's attention, the kernel injects pair p's
Q/K projection for chunk j+1, V tile p for chunk j+1, and output-projection
tile p of chunk j-1.

Attention uses the transposed layout S^T[k,q] = K @ Q^T with the two heads of
a pair row-packed via tile_position (0,0)/(64,0) (auto from base_partition) so
both K=64 score matmuls run concurrently in the PE array. attn@V is flipped:
queries ride the stationary operand (lhsT = exp-tile slice [128k x 128q]) and
V streams as the moving operand (N=65: 64 dims + ones column), halving attn@V
streaming cycles vs the [65, 512] orientation, and landing the output as
[q, dim] with the softmax denominator as a per-partition column -> the
normalize is one reciprocal [128,4] + one broadcast multiply per head, no
cross-partition broadcast needed. A PE transpose per 128q-slice (identity
rhs, PSUM fp16 out) restores [dim, q] for the output projection.
Causality: block skip + column restriction + one triangular strip mask.
Diagonal-tile exps run as one strided [128, 2, w] activation. attn@V matmuls
for key-tile i are deferred past tile i+1's scores so the PE never stalls on
the tail exps. Dummy matmuls on a zeroed tile warm the PE clock gate during
the prologue DMAs.
"""

import re

import numpy as np

import concourse.bass as bass
import concourse.tile as tile
from concourse import bacc, mybir
from concourse.bass import ts
from concourse.bass_utils import run_bass_kernel_spmd
from concourse import dve_ops as _dve_ops
from concourse.dve_spec import C0 as _C0
from concourse.dve_spec import C1 as _C1
from concourse.dve_spec import Spec as _Spec
from concourse.dve_spec import Src0 as _Src0
from concourse.dve_spec import sq as _sq

F32 = mybir.dt.float32
F16 = mybir.dt.float16

# exp(z - c) ~= ((z + 32 - c)^2 / 2048 + 0.5)^32 (2nd-order (1+u)^n), exact
# to ~0.5% on the softmax-relevant logit range. Q is pre-scaled by
# LAM = 0.125/sqrt(2048) so the PSUM score is s' = z/sqrt(2048) and the DVE
# op is exactly 8 ALU stages: sq^5(sq(s' + EC0) + 0.5).
SQ2048 = float(np.sqrt(2048.0))
EBIAS = 3.0
LAM = 0.125 / SQ2048
EC0 = (32.0 - EBIAS) / SQ2048


def _ref_expq(in0, in1, c0, c1, c2):
    t = (in0.astype(np.float32) + c0) ** 2 + c1
    for _ in range(5):
        t = t * t
    return t


def _register_expq():
    name = "EXPQ32_MHA"
    for o in _dve_ops.OPS:
        if o.name == name:
            return o
    body = _sq(_Src0 + _C0) + _C1
    for _ in range(5):
        body = _sq(body)
    spec = _Spec(body=body, reference=_ref_expq)
    op = _dve_ops.DveOp(name, spec, subdim=False, uops_sha={})
    _dve_ops.OPS.append(op)
    _dve_ops.CUSTOM_DVE_SPECS[name] = spec
    _dve_ops._SUB_OPCODE_FOR_NAME[name] = (
        max(_dve_ops._SUB_OPCODE_FOR_NAME.values()) + 1)
    try:
        op.compile("v3")
    except ValueError as e:
        m = re.search(r'uops_sha\["v3"\]="(\w+)"', str(e))
        assert m, f"could not extract sha from: {e}"
        _dve_ops.OPS.remove(op)
        op = _dve_ops.DveOp(name, spec, subdim=False,
                            uops_sha={"v3": m.group(1)})
        _dve_ops.OPS.append(op)
        _dve_ops.CUSTOM_DVE_SPECS[name] = spec
    return op


_EXPQ = _register_expq()

B = 4
S = 2048
DM = 1024
DK = 64
N_CORES = 8
H = 8
PAIRS = 4
NKT = DM // 128   # 8 contraction tiles
NQC = S // 512    # 4 query chunks
AUG = DK + 1      # 65


def _kernel_body(ctx, tc):
    nc = tc.nc
    # host-prearranged inputs (see kernel() for layouts)
    xtr = nc.dram_tensor("xtr", [128, NKT, S], F16, kind="ExternalInput").ap()
    wqr = nc.dram_tensor("wqr", [128, PAIRS * 1024], F16, kind="ExternalInput").ap()
    wkr = nc.dram_tensor("wkr", [128, PAIRS * 1024], F16, kind="ExternalInput").ap()
    wvr = nc.dram_tensor("wvr", [128, NKT * 512], F16, kind="ExternalInput").ap()
    wor = nc.dram_tensor("wor", [128, PAIRS * DM], F16, kind="ExternalInput").ap()
    tri = nc.dram_tensor("tri", [128, 128], F16, kind="ExternalInput").ap()
    idn = nc.dram_tensor("idn", [128, 128], F16, kind="ExternalInput").ap()
    y = nc.dram_tensor("y", [S, DM], F16, kind="ExternalOutput").ap()

    outer = ctx.enter_context(tc.tile_pool(name="outer", bufs=1))
    xt_all = outer.tile([128, NKT * S], F16, tag="xall", name="xall")
    xt3 = xt_all.rearrange("p (i s) -> p i s", s=S)
    wq_sb = outer.tile([128, PAIRS * 1024], F16, tag="wq", name="wq")
    wk_sb = outer.tile([128, PAIRS * 1024], F16, tag="wk", name="wk")
    wv_sb = outer.tile([128, NKT * 512], F16, tag="wv", name="wv")
    wo_sb = outer.tile([128, PAIRS * DM], F16, tag="wo", name="wo")
    tri_sb = outer.tile([128, 128], F16, tag="tri", name="tri")
    idn_sb = outer.tile([128, 128], F16, tag="idn", name="idn")
    ones1 = outer.tile([128, 1], F16, tag="ones1", name="ones1")
    ebias = outer.tile([128, 1], F32, tag="ebias", name="ebias")
    kT = [outer.tile([128, S], F16, tag=f"kT{p}", name=f"kT{p}")
          for p in range(PAIRS)]
    v_sb = [outer.tile([128, H * AUG], F16, tag=f"v{t}", name=f"v{t}")
            for t in range(4 * NQC)]

    # prologue DMA order: the first projection matmuls stream per-i-block so
    # compute chases the DMAs; x block 0 + pair-0 W_q/W_k first.
    warm = outer.tile([128, 512], F16, tag="warm", name="warm")
    nc.vector.memset(warm[:], 0.0)
    nc.sync.dma_start(out=xt3[:, 0, ts(0, 512)], in_=xtr[:, 0, ts(0, 512)])
    nc.sync.dma_start(out=wq_sb[:, ts(0, 1024)], in_=wqr[:, ts(0, 1024)])
    nc.sync.dma_start(out=wk_sb[:, ts(0, 1024)], in_=wkr[:, ts(0, 1024)])
    for i in range(1, NKT):
        nc.sync.dma_start(out=xt3[:, i, ts(0, 512)], in_=xtr[:, i, ts(0, 512)])
    for p in range(1, PAIRS):
        nc.sync.dma_start(out=wq_sb[:, ts(p, 1024)], in_=wqr[:, ts(p, 1024)])
        nc.sync.dma_start(out=wk_sb[:, ts(p, 1024)], in_=wkr[:, ts(p, 1024)])
    nc.sync.dma_start(out=wv_sb, in_=wvr)
    for j in range(1, NQC):
        nc.sync.dma_start(out=xt3[:, :, ts(j, 512)], in_=xtr[:, :, ts(j, 512)])
    nc.sync.dma_start(out=tri_sb, in_=tri)
    nc.sync.dma_start(out=idn_sb, in_=idn)
    nc.sync.dma_start(out=wo_sb, in_=wor)
    nc.vector.memset(ones1[:], 1.0)
    nc.vector.memset(ebias[:], -EBIAS)

    qcp = ctx.enter_context(tc.tile_pool(name="qcp", bufs=3))
    ap_ = ctx.enter_context(tc.tile_pool(name="attn", bufs=4))
    rp = ctx.enter_context(tc.tile_pool(name="rp", bufs=4))
    cxp = ctx.enter_context(tc.tile_pool(name="cxp", bufs=3))
    yp = ctx.enter_context(tc.tile_pool(name="yp", bufs=3))
    ps_w = ctx.enter_context(tc.tile_pool(name="psw", bufs=2, space="PSUM"))
    ps_s = ctx.enter_context(tc.tile_pool(name="pscore", bufs=2, space="PSUM"))
    ps_o = ctx.enter_context(tc.tile_pool(name="pout", bufs=1, space="PSUM"))

    def _proj_pair(j, p, qc_list):
        xoff = j * 512
        psq = ps_w.tile([128, 512], F32, tag="ps", name="ps")
        for i in range(NKT):
            nc.tensor.matmul(psq[:],
                             wq_sb[:, p * 1024 + i * 128:p * 1024 + i * 128 + 128],
                             xt_all[:, i * S + xoff:i * S + xoff + 512],
                             start=(i == 0), stop=(i == NKT - 1))
        q_ = qcp.tile([128, 512], F16, tag=f"qc{p}", name=f"qc{p}")
        nc.vector.tensor_scalar_mul(q_[:], psq[:], LAM)
        qc_list.append(q_)
        psk = ps_w.tile([128, 512], F32, tag="ps", name="ps")
        for i in range(NKT):
            nc.tensor.matmul(psk[:],
                             wk_sb[:, p * 1024 + i * 128:p * 1024 + i * 128 + 128],
                             xt_all[:, i * S + xoff:i * S + xoff + 512],
                             start=(i == 0), stop=(i == NKT - 1))
        nc.scalar.copy(kT[p][:, ts(j, 512)], psk[:])

    def _vproj(j, tt):
        xoff = j * 512
        t = 4 * j + tt
        psv = ps_w.tile([128, 512], F32, tag="ps", name="ps")
        for i in range(NKT):
            nc.tensor.matmul(psv[:],
                             xt_all[:, i * S + xoff + tt * 128:
                                   i * S + xoff + tt * 128 + 128],
                             wv_sb[:, ts(i, 512)],
                             start=(i == 0), stop=(i == NKT - 1))
        vt = v_sb[t]
        nc.vector.tensor_copy(
            vt[:].rearrange("p (h a) -> p h a", a=AUG)[:, :, 0:DK],
            psv[:].rearrange("p (h a) -> p h a", a=DK))
        ones_col = vt[:].rearrange("p (h a) -> p h a", a=AUG)[:, :, DK]
        nc.vector.tensor_copy(ones_col, ones1[:].to_broadcast((128, H)))

    def _emit_half(cxc, jj, tt, oc, ysb, tail=False):
        t = 4 * jj + tt
        psy = ps_w.tile([128, 512], F32, tag="ps", name="ps")
        for p in range(PAIRS):
            nc.tensor.matmul(psy[:], cxc[p][:, ts(tt, 128)],
                             wo_sb[:, p * DM + oc * 512:
                                   p * DM + oc * 512 + 512],
                             start=(p == 0), stop=(p == PAIRS - 1))
        if tail and oc == 0:
            # scalar-engine copy so the two halves' copies run in parallel
            nc.scalar.copy(ysb[:, ts(oc, 512)], psy[:])
        else:
            nc.vector.tensor_copy(ysb[:, ts(oc, 512)], psy[:])
        nc.sync.dma_start(out=y[ts(t, 128), ts(oc, 512)],
                          in_=ysb[:, ts(oc, 512)])

    def _emit_tile(cxc, jj, tt, tail=False):
        ysb = yp.tile([128, DM], F16, tag="y", name="ysb")
        for oc in range(2):
            _emit_half(cxc, jj, tt, oc, ysb, tail=tail)

    # dummy matmuls on the zeroed warm tile keep the PE HAM activity
    # monitor busy while the first DMAs land, so chunk-0 runs at full clock
    for _ in range(0):
        pswm = ps_w.tile([128, 512], F32, tag="ps", name="ps")
        nc.tensor.matmul(pswm[:], warm[:, 0:128], warm[:],
                         start=True, stop=True)

    # chunk-0 projections run up front (nothing to overlap them with yet)
    qc_cur = []
    for p in range(PAIRS):
        _proj_pair(0, p, qc_cur)
    for tt in range(4):
        _vproj(0, tt)

    pending = None   # (jj, cxc) for the previous chunk's output projection
    carry = None     # (pa, pi, pc0, poa, pob, ha, hb, p, jd, cxl)
    txp = None       # (cxq, p, cxl) deferred transpose+copy

    def _attn_v(pa, pi, pc0, poa, pob, ha, hb, jd):
        # flipped attn@V: lhsT = exp-tile q-slice [128 keys, 128 q],
        # rhs = per-head V (+ones) [128 keys, 65] moving; out [128 q, 65]
        # per q-slice with the denominator in column 64.
        d0 = max(pc0 // 128, 0)
        for hs, po, h in ((0, poa, ha), (1, pob, hb)):
            for qs in range(d0, 4):
                # start only on the tensor's first matmul of the round:
                # start_tensor_calc pending-zeros the WHOLE tensor, so a
                # second start=True would wipe sibling regions' has_written
                nc.tensor.matmul(
                    po[:, qs * AUG:qs * AUG + AUG],
                    pa[:, hs * 512 + qs * 128:hs * 512 + qs * 128 + 128],
                    v_sb[pi][:, h * AUG:(h + 1) * AUG],
                    start=(pi == 0 and qs == d0), stop=(pi == 4 * jd + qs))

    def _drain(c, defer=False):
        # last key-tile's attn@V, then normalize + transpose. All po reads
        # happen here (before the next pair's first attn@V write reuses the
        # single-buffered oa/ob PSUM slots). Head a's DVE normalize runs
        # under head b's attn@V stream on the PE.
        pa_, pi_, pc0_, poa_, pob_, ha_, hb_, p_, jd, cxl = c
        d0 = max(pc0_ // 128, 0)
        cxq = cxp.tile([128, 512], F16, tag="cxq", name="cxq")

        def _norm(hs, po):
            # per-partition denominators: reciprocal of po[:, 64::65], one
            # broadcast multiply into the [q, 2h*64] staging tile
            r = rp.tile([128, 4], F32, tag="r", name="r")
            den = bass.AP(tensor=po.tensor, offset=po.offset + DK,
                          ap=[list(po.ap[0]), [AUG, 4]])
            nc.vector.reciprocal_approx_fast(r[:], den)
            po3 = bass.AP(tensor=po.tensor, offset=po.offset,
                          ap=[list(po.ap[0]), [AUG, 4], [1, DK]])
            cx3 = bass.AP(tensor=cxq.tensor, offset=cxq.offset + hs * DK,
                          ap=[list(cxq.ap[0]), [128, 4], [1, DK]])
            rb = bass.AP(tensor=r.tensor, offset=r.offset,
                         ap=[list(r.ap[0]), [1, 4], [0, DK]])
            nc.vector.tensor_mul(cx3, po3, rb)

        for qs in range(d0, 4):
            nc.tensor.matmul(
                poa_[:, qs * AUG:qs * AUG + AUG],
                pa_[:, qs * 128:qs * 128 + 128],
                v_sb[pi_][:, ha_ * AUG:(ha_ + 1) * AUG],
                start=(pi_ == 0 and qs == d0), stop=(pi_ == 4 * jd + qs))
        _norm(0, poa_)
        for qs in range(d0, 4):
            nc.tensor.matmul(
                pob_[:, qs * AUG:qs * AUG + AUG],
                pa_[:, 512 + qs * 128:512 + qs * 128 + 128],
                v_sb[pi_][:, hb_ * AUG:(hb_ + 1) * AUG],
                start=(pi_ == 0 and qs == d0), stop=(pi_ == 4 * jd + qs))
        _norm(1, pob_)
        if defer:
            # transposes deferred 2 iterations into the next pair so its
            # score matmuls hide the DVE normalize latency (the PE FIFO
            # would otherwise stall on cxq at the pair boundary)
            return (cxq, p_, cxl)
        _drain_fin(cxq, p_, cxl, in_oa=True)
        return None

    def _drain_fin(cxq, p_, cxl, in_oa):
        # PE transposes [q, dim] -> [dim, q] for the output projection.
        # Immediate fins ride the oa PSUM slot (fully read by now); deferred
        # fins use a ps_w slot instead (the next pair's attn@V reclaims oa
        # at iteration 1, before a deferred transpose would write it).
        if in_oa:
            tp = ps_o.tile([128, 512], F16, tag="oa", name="tp",
                           padded_shape=[128, 512])
        else:
            tp = ps_w.tile([128, 512], F16, tag="ps", name="ps")
        for qs in range(4):
            nc.tensor.transpose(tp[:, ts(qs, 128)], cxq[:, ts(qs, 128)],
                                idn_sb[:])
        cxT = cxp.tile([128, 512], F16, tag=f"cx{p_}", name=f"cx{p_}")
        nc.vector.tensor_copy(cxT[:], tp[:])
        cxl.append(cxT)

    for j in range(NQC):
        cx_list = []
        qc_next = []
        if 1 <= j and j + 1 < NQC:
            # boundary filler: V tiles 0-1 of chunk j+1 (inputs resident
            # since the prologue) keep the PE busy while the previous
            # chunk's final exps free the score-PSUM buffers
            _vproj(j + 1, 0)
            _vproj(j + 1, 1)

        for p in range(PAIRS):
            ha, hb = 2 * p, 2 * p + 1
            nk = 4 * j + 4
            poa = ps_o.tile([128, 4 * AUG], F32, tag="oa", name="oa",
                            padded_shape=[128, 512])
            pob = ps_o.tile([128, 4 * AUG], F32, tag="ob", name="ob",
                            padded_shape=[128, 512])
            # last chunk has no next-chunk projection to inject, so spread the
            # previous chunk's output-projection tile one matmul per key-tile
            fine_emit = (j + 1 == NQC and pending is not None)
            if fine_emit:
                jj0, cxc0 = pending
                ysb_cur = yp.tile([128, DM], F16, tag="y", name="ysb")
                psys = [ps_w.tile([128, 512], F32, tag="ps", name="ps")
                        for _ in range(2)]
                emit_ops = [('mm', oc, pp) for oc in range(2)
                            for pp in range(PAIRS)]
                emit_ops.insert(4, ('fin', 0, None))
                emit_ops.append(('fin', 1, None))
                # pair-start filler: the first three accumulation matmuls
                # depend only on long-ready cx pairs 0-2 of chunk j-1;
                # they bridge the pair-transition window where only the
                # previous pair's drain matmuls are otherwise available
                for _ in range(3):
                    kind, oc, pp = emit_ops.pop(0)
                    nc.tensor.matmul(
                        psys[oc][:], cxc0[pp][:, ts(p, 128)],
                        wo_sb[:, pp * DM + oc * 512:
                              pp * DM + oc * 512 + 512],
                        start=(pp == 0), stop=(pp == PAIRS - 1))
            prev = None
            for i in range(nk):
                d = i - 4 * j
                c0 = 128 * d if d > 0 else 0
                w = 512 - c0
                at = ap_.tile([128, 1024], F16, tag="at", name="at")
                sp = ps_s.tile([128, 1024], F32, tag="sp", name="sp")
                nc.tensor.matmul(sp[0:128, c0:512],
                                 kT[p][0:64, ts(i, 128)],
                                 qc_cur[p][0:64, bass.ds(c0, w)],
                                 start=True, stop=True)
                nc.tensor.matmul(sp[0:128, 512 + c0:1024],
                                 kT[p][64:128, ts(i, 128)],
                                 qc_cur[p][64:128, bass.ds(c0, w)],
                                 start=True, stop=True)
                if carry is not None:
                    _drain(carry)
                    carry = None
                if c0 == 0:
                    if d < 0 and i % 3 == 1:
                        # offload ~1/3 of full-tile exps to the DVE via the
                        # custom 8-stage quadratic-(1+u)^32 approximation
                        nc.vector._custom_dve(_EXPQ, out=at[:], in0=sp[:],
                                              s0=EC0, s1=0.5, imm2=0.0)
                    else:
                        nc.scalar.activation(at[:], sp[:],
                                             mybir.ActivationFunctionType.Exp,
                                             bias=ebias[:], scale=SQ2048)
                else:
                    sp_strip = bass.AP(tensor=sp.tensor, offset=sp.offset + c0,
                                       ap=[list(sp.ap[0]), [512, 2], [1, w]])
                    at_strip = bass.AP(tensor=at.tensor, offset=at.offset + c0,
                                       ap=[list(at.ap[0]), [512, 2], [1, w]])
                    nc.scalar.activation(at_strip, sp_strip,
                                         mybir.ActivationFunctionType.Exp,
                                         bias=ebias[:], scale=SQ2048)
                if d >= 0:
                    strip = bass.AP(tensor=at.tensor, offset=at.offset + c0,
                                    ap=[list(at.ap[0]), [512, 2], [1, 128]])
                    tri_b = bass.AP(tensor=tri_sb.tensor, offset=tri_sb.offset,
                                    ap=[list(tri_sb.ap[0]), [0, 2], [1, 128]])
                    nc.vector.tensor_mul(strip, strip, tri_b)
                if prev is not None:
                    pa, pi, pc0 = prev
                    _attn_v(pa, pi, pc0, poa, pob, ha, hb, j)
                if fine_emit and i >= 4 and emit_ops:
                    kind, oc, pp = emit_ops.pop(0)
                    if kind == 'mm':
                        nc.tensor.matmul(
                            psys[oc][:], cxc0[pp][:, ts(p, 128)],
                            wo_sb[:, pp * DM + oc * 512:
                                  pp * DM + oc * 512 + 512],
                            start=(pp == 0), stop=(pp == PAIRS - 1))
                    else:
                        nc.vector.tensor_copy(ysb_cur[:, ts(oc, 512)],
                                              psys[oc][:])
                        nc.sync.dma_start(
                            out=y[ts(4 * jj0 + p, 128), ts(oc, 512)],
                            in_=ysb_cur[:, ts(oc, 512)])
                prev = (at, i, c0)
            pa, pi, pc0 = prev
            carry = (pa, pi, pc0, poa, pob, ha, hb, p, j, cx_list)
            # interleave next-chunk projections and previous-chunk output
            # projection into the ACT-paced attention stretch
            if j + 1 < NQC:
                _proj_pair(j + 1, p, qc_next)
                if j == 0:
                    _vproj(j + 1, p)
                elif p < PAIRS - 2:
                    _vproj(j + 1, p + 2)
            if pending is not None and not fine_emit:
                _emit_tile(pending[1], pending[0], p)
        _drain(carry)
        carry = None

        pending = (j, cx_list)
        qc_cur = qc_next
        if j == NQC - 1:
            for tt in range(4):
                _emit_tile(cx_list, j, tt, tail=True)

_NC_CACHE = None


def _build():
    global _NC_CACHE
    if _NC_CACHE is None:
        from contextlib import ExitStack
        nc = bacc.Bacc("TRN2", target_bir_lowering=False, debug=False,
                       num_devices=N_CORES)
        with tile.TileContext(nc) as tc:
            with ExitStack() as ctx:
                _kernel_body(ctx, tc)
        nc.compile()
        _NC_CACHE = nc
    return _NC_CACHE


def _make_tri():
    K = np.arange(128)[:, None]
    Q = np.arange(128)[None, :]
    return (Q >= K).astype(np.float16)


def kernel(x, W_q, W_k, W_v, W_o, _trace=False, _tmpdir=None):
    x = np.asarray(x, dtype=np.float32)
    tri = _make_tri()
    idn = np.eye(128, dtype=np.float16)
    f16 = np.float16

    def _wblk_i(W, rows):
        # i-major: [128, 8*512] with contraction block i at cols i*512..
        wT = np.ascontiguousarray(np.asarray(W)[rows, :].T)  # [1024, 512]
        return np.ascontiguousarray(
            wT.reshape(NKT, 128, 512).transpose(1, 0, 2).reshape(128, NKT * 512)
        ).astype(f16)

    def _wblk_p(W, rows):
        # pair-major: [128, p*1024 + i*128 + c]
        wT = np.ascontiguousarray(np.asarray(W)[rows, :].T)  # [1024, 512]
        return np.ascontiguousarray(
            wT.reshape(NKT, 128, PAIRS, 128).transpose(1, 2, 0, 3).reshape(
                128, PAIRS * 1024)).astype(f16)

    in_maps = []
    for c in range(N_CORES):
        b, g = divmod(c, 2)
        rows = slice(512 * g, 512 * (g + 1))
        xT = np.ascontiguousarray(x[b].T)  # [1024, 2048]
        xtr = np.ascontiguousarray(
            xT.reshape(NKT, 128, S).transpose(1, 0, 2)).astype(f16)
        woT = np.ascontiguousarray(np.asarray(W_o)[:, rows].T)  # [512, 1024]
        wor = np.ascontiguousarray(
            woT.reshape(PAIRS, 128, DM).transpose(1, 0, 2).reshape(
                128, PAIRS * DM)).astype(f16)
        in_maps.append({
            "xtr": xtr,
            "wqr": _wblk_p(W_q, rows),
            "wkr": _wblk_p(W_k, rows),
            "wvr": _wblk_i(W_v, rows),
            "wor": wor,
            "tri": tri,
            "idn": idn,
        })
    nc = _build()
    res = run_bass_kernel_spmd(nc, in_maps, core_ids=list(range(N_CORES)),
                               trace=_trace, tmpdir=_tmpdir)
    out = np.stack([res.results[2 * b]["y"].astype(np.float32)
                    + res.results[2 * b + 1]["y"].astype(np.float32)
                    for b in range(B)])
    kernel._last_exec_time_ns = res.exec_time_ns
    kernel._last_results = res
    return out


# revision 35
# speedup vs baseline: 1.0137x; 1.0137x over previous
"""Multi-head self-attention (b=4, s=2048, d_model=1024, h=16, causal) on 8 trn2 cores.

Sharding: core c = (batch b = c//2, head-group g = c%2): 8 heads of one batch
per core, full QKV + causal attention + partial W_o projection on device; host
pre-transposes x/W slices and sums the two partial y's per batch (the W_o
all-reduce done at unshard time).

All matmul operands are fp16 (full PE stream rate) with fp32 PSUM
accumulation. x and the weights arrive as host-prearranged [128, ...] DRAM
tensors -> one or four DMAs each, ordered so the first projection matmul only
waits on chunk-0 x + pair-0 W_q/W_k.

The exp stream is split between the scalar engine (ACT table exp, with the
softmax scale/bias folded into the activation's free affine) and a custom
8-stage DVE op for ~1/3 of the full key-tiles: Q is pre-scaled by
0.125/sqrt(2048) in its PSUM->SBUF copy so exp(z-3) factors as
sq^5(sq(s' + C0) + 0.5) = ((z+29)^2/2048 + 0.5)^32 -- a second-order
(1+u)^n expansion, ~0.2% end-to-end error, registered at import via the
dve_ops extension list. Other work is interleaved INTO the attention
stretch: after pair p of chunk j's attention, the kernel injects pair p's
Q/K projection for chunk j+1, V tile p for chunk j+1, and output-projection
tile p of chunk j-1.

Attention uses the transposed layout S^T[k,q] = K @ Q^T with the two heads of
a pair row-packed via tile_position (0,0)/(64,0) (auto from base_partition) so
both K=64 score matmuls run concurrently in the PE array. attn@V is flipped:
queries ride the stationary operand (lhsT = exp-tile slice [128k x 128q]) and
V streams as the moving operand (N=65: 64 dims + ones column), halving attn@V
streaming cycles vs the [65, 512] orientation, and landing the output as
[q, dim] with the softmax denominator as a per-partition column -> the
normalize is one reciprocal [128,4] + one broadcast multiply per head, no
cross-partition broadcast needed. A PE transpose per 128q-slice (identity
rhs, PSUM fp16 out) restores [dim, q] for the output projection.
Causality: block skip + column restriction + one triangular strip mask.
Diagonal-tile exps run as one strided [128, 2, w] activation. attn@V matmuls
for key-tile i are deferred past tile i+1's scores so the PE never stalls on
the tail exps. Dummy matmuls on a zeroed tile warm the PE clock gate during
the prologue DMAs.
"""

import re

import numpy as np

import concourse.bass as bass
import concourse.tile as tile
from concourse import bacc, mybir
from concourse.bass import ts
from concourse.bass_utils import run_bass_kernel_spmd
from concourse import dve_ops as _dve_ops
from concourse.dve_spec import C0 as _C0
from concourse.dve_spec import C1 as _C1
from concourse.dve_spec import Spec as _Spec
from concourse.dve_spec import Src0 as _Src0
from concourse.dve_spec import sq as _sq

F32 = mybir.dt.float32
F16 = mybir.dt.float16

# exp(z - c) ~= ((z + 32 - c)^2 / 2048 + 0.5)^32 (2nd-order (1+u)^n), exact
# to ~0.5% on the softmax-relevant logit range. Q is pre-scaled by
# LAM = 0.125/sqrt(2048) so the PSUM score is s' = z/sqrt(2048) and the DVE
# op is exactly 8 ALU stages: sq^5(sq(s' + EC0) + 0.5).
SQ2048 = float(np.sqrt(2048.0))
EBIAS = 3.0
LAM = 0.125 / SQ2048
EC0 = (32.0 - EBIAS) / SQ2048


def _ref_expq(in0, in1, c0, c1, c2):
    t = (in0.astype(np.float32) + c0) ** 2 + c1
    for _ in range(5):
        t = t * t
    return t


def _register_expq():
    name = "EXPQ32_MHA"
    for o in _dve_ops.OPS:
        if o.name == name:
            return o
    body = _sq(_Src0 + _C0) + _C1
    for _ in range(5):
        body = _sq(body)
    spec = _Spec(body=body, reference=_ref_expq)
    op = _dve_ops.DveOp(name, spec, subdim=False, uops_sha={})
    _dve_ops.OPS.append(op)
    _dve_ops.CUSTOM_DVE_SPECS[name] = spec
    _dve_ops._SUB_OPCODE_FOR_NAME[name] = (
        max(_dve_ops._SUB_OPCODE_FOR_NAME.values()) + 1)
    try:
        op.compile("v3")
    except ValueError as e:
        m = re.search(r'uops_sha\["v3"\]="(\w+)"', str(e))
        assert m, f"could not extract sha from: {e}"
        _dve_ops.OPS.remove(op)
        op = _dve_ops.DveOp(name, spec, subdim=False,
                            uops_sha={"v3": m.group(1)})
        _dve_ops.OPS.append(op)
        _dve_ops.CUSTOM_DVE_SPECS[name] = spec
    return op


_EXPQ = _register_expq()

B = 4
S = 2048
DM = 1024
DK = 64
N_CORES = 8
H = 8
PAIRS = 4
NKT = DM // 128   # 8 contraction tiles
NQC = S // 512    # 4 query chunks
AUG = DK + 1      # 65


def _kernel_body(ctx, tc):
    nc = tc.nc
    # host-prearranged inputs (see kernel() for layouts)
    xtr = nc.dram_tensor("xtr", [128, NKT, S], F16, kind="ExternalInput").ap()
    wqr = nc.dram_tensor("wqr", [128, PAIRS * 1024], F16, kind="ExternalInput").ap()
    wkr = nc.dram_tensor("wkr", [128, PAIRS * 1024], F16, kind="ExternalInput").ap()
    wvr = nc.dram_tensor("wvr", [128, NKT * 512], F16, kind="ExternalInput").ap()
    wor = nc.dram_tensor("wor", [128, PAIRS * DM], F16, kind="ExternalInput").ap()
    tri = nc.dram_tensor("tri", [128, 128], F16, kind="ExternalInput").ap()
    idn = nc.dram_tensor("idn", [128, 128], F16, kind="ExternalInput").ap()
    y = nc.dram_tensor("y", [S, DM], F16, kind="ExternalOutput").ap()

    outer = ctx.enter_context(tc.tile_pool(name="outer", bufs=1))
    xt_all = outer.tile([128, NKT * S], F16, tag="xall", name="xall")
    xt3 = xt_all.rearrange("p (i s) -> p i s", s=S)
    wq_sb = outer.tile([128, PAIRS * 1024], F16, tag="wq", name="wq")
    wk_sb = outer.tile([128, PAIRS * 1024], F16, tag="wk", name="wk")
    wv_sb = outer.tile([128, NKT * 512], F16, tag="wv", name="wv")
    wo_sb = outer.tile([128, PAIRS * DM], F16, tag="wo", name="wo")
    tri_sb = outer.tile([128, 128], F16, tag="tri", name="tri")
    idn_sb = outer.tile([128, 128], F16, tag="idn", name="idn")
    ones1 = outer.tile([128, 1], F16, tag="ones1", name="ones1")
    ebias = outer.tile([128, 1], F32, tag="ebias", name="ebias")
    kT = [outer.tile([128, S], F16, tag=f"kT{p}", name=f"kT{p}")
          for p in range(PAIRS)]
    v_sb = [outer.tile([128, H * AUG], F16, tag=f"v{t}", name=f"v{t}")
            for t in range(4 * NQC)]

    # prologue DMA order: the first projection matmuls stream per-i-block so
    # compute chases the DMAs; x block 0 + pair-0 W_q/W_k first.
    warm = outer.tile([128, 512], F16, tag="warm", name="warm")
    nc.vector.memset(warm[:], 0.0)
    nc.sync.dma_start(out=xt3[:, 0, ts(0, 512)], in_=xtr[:, 0, ts(0, 512)])
    nc.sync.dma_start(out=wq_sb[:, ts(0, 1024)], in_=wqr[:, ts(0, 1024)])
    nc.sync.dma_start(out=wk_sb[:, ts(0, 1024)], in_=wkr[:, ts(0, 1024)])
    for i in range(1, NKT):
        nc.sync.dma_start(out=xt3[:, i, ts(0, 512)], in_=xtr[:, i, ts(0, 512)])
    for p in range(1, PAIRS):
        nc.sync.dma_start(out=wq_sb[:, ts(p, 1024)], in_=wqr[:, ts(p, 1024)])
        nc.sync.dma_start(out=wk_sb[:, ts(p, 1024)], in_=wkr[:, ts(p, 1024)])
    nc.sync.dma_start(out=wv_sb, in_=wvr)
    for j in range(1, NQC):
        nc.sync.dma_start(out=xt3[:, :, ts(j, 512)], in_=xtr[:, :, ts(j, 512)])
    nc.sync.dma_start(out=tri_sb, in_=tri)
    nc.sync.dma_start(out=idn_sb, in_=idn)
    nc.sync.dma_start(out=wo_sb, in_=wor)
    nc.vector.memset(ones1[:], 1.0)
    nc.vector.memset(ebias[:], -EBIAS)

    qcp = ctx.enter_context(tc.tile_pool(name="qcp", bufs=3))
    ap_ = ctx.enter_context(tc.tile_pool(name="attn", bufs=4))
    rp = ctx.enter_context(tc.tile_pool(name="rp", bufs=4))
    cxp = ctx.enter_context(tc.tile_pool(name="cxp", bufs=3))
    yp = ctx.enter_context(tc.tile_pool(name="yp", bufs=3))
    ps_w = ctx.enter_context(tc.tile_pool(name="psw", bufs=2, space="PSUM"))
    ps_s = ctx.enter_context(tc.tile_pool(name="pscore", bufs=2, space="PSUM"))
    ps_o = ctx.enter_context(tc.tile_pool(name="pout", bufs=1, space="PSUM"))

    def _proj_pair(j, p, qc_list):
        xoff = j * 512
        psq = ps_w.tile([128, 512], F32, tag="ps", name="ps")
        for i in range(NKT):
            nc.tensor.matmul(psq[:],
                             wq_sb[:, p * 1024 + i * 128:p * 1024 + i * 128 + 128],
                             xt_all[:, i * S + xoff:i * S + xoff + 512],
                             start=(i == 0), stop=(i == NKT - 1))
        q_ = qcp.tile([128, 512], F16, tag=f"qc{p}", name=f"qc{p}")
        nc.vector.tensor_scalar_mul(q_[:], psq[:], LAM)
        qc_list.append(q_)
        psk = ps_w.tile([128, 512], F32, tag="ps", name="ps")
        for i in range(NKT):
            nc.tensor.matmul(psk[:],
                             wk_sb[:, p * 1024 + i * 128:p * 1024 + i * 128 + 128],
                             xt_all[:, i * S + xoff:i * S + xoff + 512],
                             start=(i == 0), stop=(i == NKT - 1))
        nc.scalar.copy(kT[p][:, ts(j, 512)], psk[:])

    def _vproj(j, tt):
        xoff = j * 512
        t = 4 * j + tt
        psv = ps_w.tile([128, 512], F32, tag="ps", name="ps")
        for i in range(NKT):
            nc.tensor.matmul(psv[:],
                             xt_all[:, i * S + xoff + tt * 128:
                                   i * S + xoff + tt * 128 + 128],
                             wv_sb[:, ts(i, 512)],
                             start=(i == 0), stop=(i == NKT - 1))
        vt = v_sb[t]
        nc.vector.tensor_copy(
            vt[:].rearrange("p (h a) -> p h a", a=AUG)[:, :, 0:DK],
            psv[:].rearrange("p (h a) -> p h a", a=DK))
        ones_col = vt[:].rearrange("p (h a) -> p h a", a=AUG)[:, :, DK]
        nc.vector.tensor_copy(ones_col, ones1[:].to_broadcast((128, H)))

    def _emit_half(cxc, jj, tt, oc, ysb, tail=False):
        t = 4 * jj + tt
        psy = ps_w.tile([128, 512], F32, tag="ps", name="ps")
        for p in range(PAIRS):
            nc.tensor.matmul(psy[:], cxc[p][:, ts(tt, 128)],
                             wo_sb[:, p * DM + oc * 512:
                                   p * DM + oc * 512 + 512],
                             start=(p == 0), stop=(p == PAIRS - 1))
        if tail and oc == 0:
            # scalar-engine copy so the two halves' copies run in parallel
            nc.scalar.copy(ysb[:, ts(oc, 512)], psy[:])
        else:
            nc.vector.tensor_copy(ysb[:, ts(oc, 512)], psy[:])
        nc.sync.dma_start(out=y[ts(t, 128), ts(oc, 512)],
                          in_=ysb[:, ts(oc, 512)])

    def _emit_tile(cxc, jj, tt, tail=False):
        ysb = yp.tile([128, DM], F16, tag="y", name="ysb")
        for oc in range(2):
            _emit_half(cxc, jj, tt, oc, ysb, tail=tail)

    # dummy matmuls on the zeroed warm tile keep the PE HAM activity
    # monitor busy while the first DMAs land, so chunk-0 runs at full clock
    for _ in range(40):
        pswm = ps_w.tile([128, 512], F32, tag="ps", name="ps")
        nc.tensor.matmul(pswm[:, 0:128], warm[:, 0:128], warm[:, 0:128],
                         start=True, stop=True)

    # chunk-0 projections run up front (nothing to overlap them with yet)
    qc_cur = []
    for p in range(PAIRS):
        _proj_pair(0, p, qc_cur)
    for tt in range(4):
        _vproj(0, tt)

    pending = None   # (jj, cxc) for the previous chunk's output projection
    carry = None     # (pa, pi, pc0, poa, pob, ha, hb, p, jd, cxl)
    txp = None       # (cxq, p, cxl) deferred transpose+copy

    def _attn_v(pa, pi, pc0, poa, pob, ha, hb, jd):
        # flipped attn@V: lhsT = exp-tile q-slice [128 keys, 128 q],
        # rhs = per-head V (+ones) [128 keys, 65] moving; out [128 q, 65]
        # per q-slice with the denominator in column 64.
        d0 = max(pc0 // 128, 0)
        for hs, po, h in ((0, poa, ha), (1, pob, hb)):
            for qs in range(d0, 4):
                # start only on the tensor's first matmul of the round:
                # start_tensor_calc pending-zeros the WHOLE tensor, so a
                # second start=True would wipe sibling regions' has_written
                nc.tensor.matmul(
                    po[:, qs * AUG:qs * AUG + AUG],
                    pa[:, hs * 512 + qs * 128:hs * 512 + qs * 128 + 128],
                    v_sb[pi][:, h * AUG:(h + 1) * AUG],
                    start=(pi == 0 and qs == d0), stop=(pi == 4 * jd + qs))

    def _drain(c, defer=False):
        # last key-tile's attn@V, then normalize + transpose. All po reads
        # happen here (before the next pair's first attn@V write reuses the
        # single-buffered oa/ob PSUM slots). Head a's DVE normalize runs
        # under head b's attn@V stream on the PE.
        pa_, pi_, pc0_, poa_, pob_, ha_, hb_, p_, jd, cxl = c
        d0 = max(pc0_ // 128, 0)
        cxq = cxp.tile([128, 512], F16, tag="cxq", name="cxq")

        def _norm(hs, po):
            # per-partition denominators: reciprocal of po[:, 64::65], one
            # broadcast multiply into the [q, 2h*64] staging tile
            r = rp.tile([128, 4], F32, tag="r", name="r")
            den = bass.AP(tensor=po.tensor, offset=po.offset + DK,
                          ap=[list(po.ap[0]), [AUG, 4]])
            nc.vector.reciprocal_approx_fast(r[:], den)
            po3 = bass.AP(tensor=po.tensor, offset=po.offset,
                          ap=[list(po.ap[0]), [AUG, 4], [1, DK]])
            cx3 = bass.AP(tensor=cxq.tensor, offset=cxq.offset + hs * DK,
                          ap=[list(cxq.ap[0]), [128, 4], [1, DK]])
            rb = bass.AP(tensor=r.tensor, offset=r.offset,
                         ap=[list(r.ap[0]), [1, 4], [0, DK]])
            nc.vector.tensor_mul(cx3, po3, rb)

        for qs in range(d0, 4):
            nc.tensor.matmul(
                poa_[:, qs * AUG:qs * AUG + AUG],
                pa_[:, qs * 128:qs * 128 + 128],
                v_sb[pi_][:, ha_ * AUG:(ha_ + 1) * AUG],
                start=(pi_ == 0 and qs == d0), stop=(pi_ == 4 * jd + qs))
        _norm(0, poa_)
        for qs in range(d0, 4):
            nc.tensor.matmul(
                pob_[:, qs * AUG:qs * AUG + AUG],
                pa_[:, 512 + qs * 128:512 + qs * 128 + 128],
                v_sb[pi_][:, hb_ * AUG:(hb_ + 1) * AUG],
                start=(pi_ == 0 and qs == d0), stop=(pi_ == 4 * jd + qs))
        _norm(1, pob_)
        if defer:
            # transposes deferred 2 iterations into the next pair so its
            # score matmuls hide the DVE normalize latency (the PE FIFO
            # would otherwise stall on cxq at the pair boundary)
            return (cxq, p_, cxl)
        _drain_fin(cxq, p_, cxl, in_oa=True)
        return None

    def _drain_fin(cxq, p_, cxl, in_oa):
        # PE transposes [q, dim] -> [dim, q] for the output projection.
        # Immediate fins ride the oa PSUM slot (fully read by now); deferred
        # fins use a ps_w slot instead (the next pair's attn@V reclaims oa
        # at iteration 1, before a deferred transpose would write it).
        if in_oa:
            tp = ps_o.tile([128, 512], F16, tag="oa", name="tp",
                           padded_shape=[128, 512])
        else:
            tp = ps_w.tile([128, 512], F16, tag="ps", name="ps")
        for qs in range(4):
            nc.tensor.transpose(tp[:, ts(qs, 128)], cxq[:, ts(qs, 128)],
                                idn_sb[:])
        cxT = cxp.tile([128, 512], F16, tag=f"cx{p_}", name=f"cx{p_}")
        nc.vector.tensor_copy(cxT[:], tp[:])
        cxl.append(cxT)

    for j in range(NQC):
        cx_list = []
        qc_next = []
        if 1 <= j and j + 1 < NQC:
            # boundary filler: V tiles 0-1 of chunk j+1 (inputs resident
            # since the prologue) keep the PE busy while the previous
            # chunk's final exps free the score-PSUM buffers
            _vproj(j + 1, 0)
            _vproj(j + 1, 1)

        for p in range(PAIRS):
            ha, hb = 2 * p, 2 * p + 1
            nk = 4 * j + 4
            poa = ps_o.tile([128, 4 * AUG], F32, tag="oa", name="oa",
                            padded_shape=[128, 512])
            pob = ps_o.tile([128, 4 * AUG], F32, tag="ob", name="ob",
                            padded_shape=[128, 512])
            # last chunk has no next-chunk projection to inject, so spread the
            # previous chunk's output-projection tile one matmul per key-tile
            fine_emit = (j + 1 == NQC and pending is not None)
            if fine_emit:
                jj0, cxc0 = pending
                ysb_cur = yp.tile([128, DM], F16, tag="y", name="ysb")
                psys = [ps_w.tile([128, 512], F32, tag="ps", name="ps")
                        for _ in range(2)]
                emit_ops = [('mm', oc, pp) for oc in range(2)
                            for pp in range(PAIRS)]
                emit_ops.insert(4, ('fin', 0, None))
                emit_ops.append(('fin', 1, None))
                # pair-start filler: the first three accumulation matmuls
                # depend only on long-ready cx pairs 0-2 of chunk j-1;
                # they bridge the pair-transition window where only the
                # previous pair's drain matmuls are otherwise available
                for _ in range(3):
                    kind, oc, pp = emit_ops.pop(0)
                    nc.tensor.matmul(
                        psys[oc][:], cxc0[pp][:, ts(p, 128)],
                        wo_sb[:, pp * DM + oc * 512:
                              pp * DM + oc * 512 + 512],
                        start=(pp == 0), stop=(pp == PAIRS - 1))
            prev = None
            for i in range(nk):
                d = i - 4 * j
                c0 = 128 * d if d > 0 else 0
                w = 512 - c0
                at = ap_.tile([128, 1024], F16, tag="at", name="at")
                sp = ps_s.tile([128, 1024], F32, tag="sp", name="sp")
                nc.tensor.matmul(sp[0:128, c0:512],
                                 kT[p][0:64, ts(i, 128)],
                                 qc_cur[p][0:64, bass.ds(c0, w)],
                                 start=True, stop=True)
                nc.tensor.matmul(sp[0:128, 512 + c0:1024],
                                 kT[p][64:128, ts(i, 128)],
                                 qc_cur[p][64:128, bass.ds(c0, w)],
                                 start=True, stop=True)
                if carry is not None:
                    _drain(carry)
                    carry = None
                if c0 == 0:
                    if d < 0 and i % 3 == 1:
                        # offload ~1/3 of full-tile exps to the DVE via the
                        # custom 8-stage quadratic-(1+u)^32 approximation
                        nc.vector._custom_dve(_EXPQ, out=at[:], in0=sp[:],
                                              s0=EC0, s1=0.5, imm2=0.0)
                    else:
                        nc.scalar.activation(at[:], sp[:],
                                             mybir.ActivationFunctionType.Exp,
                                             bias=ebias[:], scale=SQ2048)
                else:
                    sp_strip = bass.AP(tensor=sp.tensor, offset=sp.offset + c0,
                                       ap=[list(sp.ap[0]), [512, 2], [1, w]])
                    at_strip = bass.AP(tensor=at.tensor, offset=at.offset + c0,
                                       ap=[list(at.ap[0]), [512, 2], [1, w]])
                    nc.scalar.activation(at_strip, sp_strip,
                                         mybir.ActivationFunctionType.Exp,
                                         bias=ebias[:], scale=SQ2048)
                if d >= 0:
                    strip = bass.AP(tensor=at.tensor, offset=at.offset + c0,
                                    ap=[list(at.ap[0]), [512, 2], [1, 128]])
                    tri_b = bass.AP(tensor=tri_sb.tensor, offset=tri_sb.offset,
                                    ap=[list(tri_sb.ap[0]), [0, 2], [1, 128]])
                    nc.vector.tensor_mul(strip, strip, tri_b)
                if prev is not None:
                    pa, pi, pc0 = prev
                    _attn_v(pa, pi, pc0, poa, pob, ha, hb, j)
                if fine_emit and i >= 4 and emit_ops:
                    kind, oc, pp = emit_ops.pop(0)
                    if kind == 'mm':
                        nc.tensor.matmul(
                            psys[oc][:], cxc0[pp][:, ts(p, 128)],
                            wo_sb[:, pp * DM + oc * 512:
                                  pp * DM + oc * 512 + 512],
                            start=(pp == 0), stop=(pp == PAIRS - 1))
                    else:
                        nc.vector.tensor_copy(ysb_cur[:, ts(oc, 512)],
                                              psys[oc][:])
                        nc.sync.dma_start(
                            out=y[ts(4 * jj0 + p, 128), ts(oc, 512)],
                            in_=ysb_cur[:, ts(oc, 512)])
                prev = (at, i, c0)
            pa, pi, pc0 = prev
            carry = (pa, pi, pc0, poa, pob, ha, hb, p, j, cx_list)
            # interleave next-chunk projections and previous-chunk output
            # projection into the ACT-paced attention stretch
            if j + 1 < NQC:
                _proj_pair(j + 1, p, qc_next)
                if j == 0:
                    _vproj(j + 1, p)
                elif p < PAIRS - 2:
                    _vproj(j + 1, p + 2)
            if pending is not None and not fine_emit:
                _emit_tile(pending[1], pending[0], p)
        _drain(carry)
        carry = None

        pending = (j, cx_list)
        qc_cur = qc_next
        if j == NQC - 1:
            for tt in range(4):
                _emit_tile(cx_list, j, tt, tail=True)

_NC_CACHE = None


def _build():
    global _NC_CACHE
    if _NC_CACHE is None:
        from contextlib import ExitStack
        nc = bacc.Bacc("TRN2", target_bir_lowering=False, debug=False,
                       num_devices=N_CORES)
        with tile.TileContext(nc) as tc:
            with ExitStack() as ctx:
                _kernel_body(ctx, tc)
        nc.compile()
        _NC_CACHE = nc
    return _NC_CACHE


def _make_tri():
    K = np.arange(128)[:, None]
    Q = np.arange(128)[None, :]
    return (Q >= K).astype(np.float16)


def kernel(x, W_q, W_k, W_v, W_o, _trace=False, _tmpdir=None):
    x = np.asarray(x, dtype=np.float32)
    tri = _make_tri()
    idn = np.eye(128, dtype=np.float16)
    f16 = np.float16

    def _wblk_i(W, rows):
        # i-major: [128, 8*512] with contraction block i at cols i*512..
        wT = np.ascontiguousarray(np.asarray(W)[rows, :].T)  # [1024, 512]
        return np.ascontiguousarray(
            wT.reshape(NKT, 128, 512).transpose(1, 0, 2).reshape(128, NKT * 512)
        ).astype(f16)

    def _wblk_p(W, rows):
        # pair-major: [128, p*1024 + i*128 + c]
        wT = np.ascontiguousarray(np.asarray(W)[rows, :].T)  # [1024, 512]
        return np.ascontiguousarray(
            wT.reshape(NKT, 128, PAIRS, 128).transpose(1, 2, 0, 3).reshape(
                128, PAIRS * 1024)).astype(f16)

    in_maps = []
    for c in range(N_CORES):
        b, g = divmod(c, 2)
        rows = slice(512 * g, 512 * (g + 1))
        xT = np.ascontiguousarray(x[b].T)  # [1024, 2048]
        xtr = np.ascontiguousarray(
            xT.reshape(NKT, 128, S).transpose(1, 0, 2)).astype(f16)
        woT = np.ascontiguousarray(np.asarray(W_o)[:, rows].T)  # [512, 1024]
        wor = np.ascontiguousarray(
            woT.reshape(PAIRS, 128, DM).transpose(1, 0, 2).reshape(
                128, PAIRS * DM)).astype(f16)
        in_maps.append({
            "xtr": xtr,
            "wqr": _wblk_p(W_q, rows),
            "wkr": _wblk_p(W_k, rows),
            "wvr": _wblk_i(W_v, rows),
            "wor": wor,
            "tri": tri,
            "idn": idn,
        })
    nc = _build()
    res = run_bass_kernel_spmd(nc, in_maps, core_ids=list(range(N_CORES)),
                               trace=_trace, tmpdir=_tmpdir)
    out = np.stack([res.results[2 * b]["y"].astype(np.float32)
                    + res.results[2 * b + 1]["y"].astype(np.float32)
                    for b in range(B)])
    kernel._last_exec_time_ns = res.exec_time_ns
    kernel._last_results = res
    return out


# revision 36
# speedup vs baseline: 1.0168x; 1.0031x over previous
"""Multi-head self-attention (b=4, s=2048, d_model=1024, h=16, causal) on 8 trn2 cores.

Sharding: core c = (batch b = c//2, head-group g = c%2): 8 heads of one batch
per core, full QKV + causal attention + partial W_o projection on device; host
pre-transposes x/W slices and sums the two partial y's per batch (the W_o
all-reduce done at unshard time).

All matmul operands are fp16 (full PE stream rate) with fp32 PSUM
accumulation. x and the weights arrive as host-prearranged [128, ...] DRAM
tensors -> one or four DMAs each, ordered so the first projection matmul only
waits on chunk-0 x + pair-0 W_q/W_k.

The exp stream is split between the scalar engine (ACT table exp, with the
softmax scale/bias folded into the activation's free affine) and a custom
8-stage DVE op for ~1/3 of the full key-tiles: Q is pre-scaled by
0.125/sqrt(2048) in its PSUM->SBUF copy so exp(z-3) factors as
sq^5(sq(s' + C0) + 0.5) = ((z+29)^2/2048 + 0.5)^32 -- a second-order
(1+u)^n expansion, ~0.2% end-to-end error, registered at import via the
dve_ops extension list. Other work is interleaved INTO the attention
stretch: after pair p of chunk j's attention, the kernel injects pair p's
Q/K projection for chunk j+1, V tile p for chunk j+1, and output-projection
tile p of chunk j-1.

Attention uses the transposed layout S^T[k,q] = K @ Q^T with the two heads of
a pair row-packed via tile_position (0,0)/(64,0) (auto from base_partition) so
both K=64 score matmuls run concurrently in the PE array. attn@V is flipped:
queries ride the stationary operand (lhsT = exp-tile slice [128k x 128q]) and
V streams as the moving operand (N=65: 64 dims + ones column), halving attn@V
streaming cycles vs the [65, 512] orientation, and landing the output as
[q, dim] with the softmax denominator as a per-partition column -> the
normalize is one reciprocal [128,4] + one broadcast multiply per head, no
cross-partition broadcast needed. A PE transpose per 128q-slice (identity
rhs, PSUM fp16 out) restores [dim, q] for the output projection.
Causality: block skip + column restriction + one triangular strip mask.
Diagonal-tile exps run as one strided [128, 2, w] activation. attn@V matmuls
for key-tile i are deferred past tile i+1's scores so the PE never stalls on
the tail exps. Dummy matmuls on a zeroed tile warm the PE clock gate during
the prologue DMAs.
"""

import re

import numpy as np

import concourse.bass as bass
import concourse.tile as tile
from concourse import bacc, mybir
from concourse.bass import ts
from concourse.bass_utils import run_bass_kernel_spmd
from concourse import dve_ops as _dve_ops
from concourse.dve_spec import C0 as _C0
from concourse.dve_spec import C1 as _C1
from concourse.dve_spec import Spec as _Spec
from concourse.dve_spec import Src0 as _Src0
from concourse.dve_spec import sq as _sq

F32 = mybir.dt.float32
F16 = mybir.dt.float16

# exp(z - c) ~= ((z + 32 - c)^2 / 2048 + 0.5)^32 (2nd-order (1+u)^n), exact
# to ~0.5% on the softmax-relevant logit range. Q is pre-scaled by
# LAM = 0.125/sqrt(2048) so the PSUM score is s' = z/sqrt(2048) and the DVE
# op is exactly 8 ALU stages: sq^5(sq(s' + EC0) + 0.5).
SQ2048 = float(np.sqrt(2048.0))
EBIAS = 3.0
LAM = 0.125 / SQ2048
EC0 = (32.0 - EBIAS) / SQ2048


def _ref_expq(in0, in1, c0, c1, c2):
    t = (in0.astype(np.float32) + c0) ** 2 + c1
    for _ in range(5):
        t = t * t
    return t


def _register_expq():
    name = "EXPQ32_MHA"
    for o in _dve_ops.OPS:
        if o.name == name:
            return o
    body = _sq(_Src0 + _C0) + _C1
    for _ in range(5):
        body = _sq(body)
    spec = _Spec(body=body, reference=_ref_expq)
    op = _dve_ops.DveOp(name, spec, subdim=False, uops_sha={})
    _dve_ops.OPS.append(op)
    _dve_ops.CUSTOM_DVE_SPECS[name] = spec
    _dve_ops._SUB_OPCODE_FOR_NAME[name] = (
        max(_dve_ops._SUB_OPCODE_FOR_NAME.values()) + 1)
    try:
        op.compile("v3")
    except ValueError as e:
        m = re.search(r'uops_sha\["v3"\]="(\w+)"', str(e))
        assert m, f"could not extract sha from: {e}"
        _dve_ops.OPS.remove(op)
        op = _dve_ops.DveOp(name, spec, subdim=False,
                            uops_sha={"v3": m.group(1)})
        _dve_ops.OPS.append(op)
        _dve_ops.CUSTOM_DVE_SPECS[name] = spec
    return op


_EXPQ = _register_expq()

B = 4
S = 2048
DM = 1024
DK = 64
N_CORES = 8
H = 8
PAIRS = 4
NKT = DM // 128   # 8 contraction tiles
NQC = S // 512    # 4 query chunks
AUG = DK + 1      # 65


def _kernel_body(ctx, tc):
    nc = tc.nc
    # host-prearranged inputs (see kernel() for layouts)
    xtr = nc.dram_tensor("xtr", [128, NKT, S], F16, kind="ExternalInput").ap()
    wqr = nc.dram_tensor("wqr", [128, PAIRS * 1024], F16, kind="ExternalInput").ap()
    wkr = nc.dram_tensor("wkr", [128, PAIRS * 1024], F16, kind="ExternalInput").ap()
    wvr = nc.dram_tensor("wvr", [128, NKT * 512], F16, kind="ExternalInput").ap()
    wor = nc.dram_tensor("wor", [128, PAIRS * DM], F16, kind="ExternalInput").ap()
    tri = nc.dram_tensor("tri", [128, 128], F16, kind="ExternalInput").ap()
    idn = nc.dram_tensor("idn", [128, 128], F16, kind="ExternalInput").ap()
    y = nc.dram_tensor("y", [S, DM], F16, kind="ExternalOutput").ap()

    outer = ctx.enter_context(tc.tile_pool(name="outer", bufs=1))
    xt_all = outer.tile([128, NKT * S], F16, tag="xall", name="xall")
    xt3 = xt_all.rearrange("p (i s) -> p i s", s=S)
    wq_sb = outer.tile([128, PAIRS * 1024], F16, tag="wq", name="wq")
    wk_sb = outer.tile([128, PAIRS * 1024], F16, tag="wk", name="wk")
    wv_sb = outer.tile([128, NKT * 512], F16, tag="wv", name="wv")
    wo_sb = outer.tile([128, PAIRS * DM], F16, tag="wo", name="wo")
    tri_sb = outer.tile([128, 128], F16, tag="tri", name="tri")
    idn_sb = outer.tile([128, 128], F16, tag="idn", name="idn")
    ones1 = outer.tile([128, 1], F16, tag="ones1", name="ones1")
    ebias = outer.tile([128, 1], F32, tag="ebias", name="ebias")
    kT = [outer.tile([128, S], F16, tag=f"kT{p}", name=f"kT{p}")
          for p in range(PAIRS)]
    v_sb = [outer.tile([128, H * AUG], F16, tag=f"v{t}", name=f"v{t}")
            for t in range(4 * NQC)]

    # prologue DMA order: the first projection matmuls stream per-i-block so
    # compute chases the DMAs; x block 0 + pair-0 W_q/W_k first.
    warm = outer.tile([128, 512], F16, tag="warm", name="warm")
    nc.vector.memset(warm[:], 0.0)
    nc.sync.dma_start(out=xt3[:, 0, ts(0, 512)], in_=xtr[:, 0, ts(0, 512)])
    nc.sync.dma_start(out=wq_sb[:, ts(0, 1024)], in_=wqr[:, ts(0, 1024)])
    nc.sync.dma_start(out=wk_sb[:, ts(0, 1024)], in_=wkr[:, ts(0, 1024)])
    for i in range(1, NKT):
        nc.sync.dma_start(out=xt3[:, i, ts(0, 512)], in_=xtr[:, i, ts(0, 512)])
    for p in range(1, PAIRS):
        nc.sync.dma_start(out=wq_sb[:, ts(p, 1024)], in_=wqr[:, ts(p, 1024)])
        nc.sync.dma_start(out=wk_sb[:, ts(p, 1024)], in_=wkr[:, ts(p, 1024)])
    nc.sync.dma_start(out=wv_sb, in_=wvr)
    for j in range(1, NQC):
        nc.sync.dma_start(out=xt3[:, :, ts(j, 512)], in_=xtr[:, :, ts(j, 512)])
    nc.sync.dma_start(out=tri_sb, in_=tri)
    nc.sync.dma_start(out=idn_sb, in_=idn)
    nc.sync.dma_start(out=wo_sb, in_=wor)
    nc.vector.memset(ones1[:], 1.0)
    nc.vector.memset(ebias[:], -EBIAS)

    qcp = ctx.enter_context(tc.tile_pool(name="qcp", bufs=3))
    ap_ = ctx.enter_context(tc.tile_pool(name="attn", bufs=4))
    rp = ctx.enter_context(tc.tile_pool(name="rp", bufs=4))
    cxp = ctx.enter_context(tc.tile_pool(name="cxp", bufs=3))
    yp = ctx.enter_context(tc.tile_pool(name="yp", bufs=3))
    ps_w = ctx.enter_context(tc.tile_pool(name="psw", bufs=2, space="PSUM"))
    ps_s = ctx.enter_context(tc.tile_pool(name="pscore", bufs=2, space="PSUM"))
    ps_o = ctx.enter_context(tc.tile_pool(name="pout", bufs=1, space="PSUM"))

    def _proj_pair(j, p, qc_list):
        xoff = j * 512
        psq = ps_w.tile([128, 512], F32, tag="ps", name="ps")
        for i in range(NKT):
            nc.tensor.matmul(psq[:],
                             wq_sb[:, p * 1024 + i * 128:p * 1024 + i * 128 + 128],
                             xt_all[:, i * S + xoff:i * S + xoff + 512],
                             start=(i == 0), stop=(i == NKT - 1))
        q_ = qcp.tile([128, 512], F16, tag=f"qc{p}", name=f"qc{p}")
        nc.vector.tensor_scalar_mul(q_[:], psq[:], LAM)
        qc_list.append(q_)
        psk = ps_w.tile([128, 512], F32, tag="ps", name="ps")
        for i in range(NKT):
            nc.tensor.matmul(psk[:],
                             wk_sb[:, p * 1024 + i * 128:p * 1024 + i * 128 + 128],
                             xt_all[:, i * S + xoff:i * S + xoff + 512],
                             start=(i == 0), stop=(i == NKT - 1))
        nc.scalar.copy(kT[p][:, ts(j, 512)], psk[:])

    def _vproj(j, tt):
        xoff = j * 512
        t = 4 * j + tt
        psv = ps_w.tile([128, 512], F32, tag="ps", name="ps")
        for i in range(NKT):
            nc.tensor.matmul(psv[:],
                             xt_all[:, i * S + xoff + tt * 128:
                                   i * S + xoff + tt * 128 + 128],
                             wv_sb[:, ts(i, 512)],
                             start=(i == 0), stop=(i == NKT - 1))
        vt = v_sb[t]
        nc.vector.tensor_copy(
            vt[:].rearrange("p (h a) -> p h a", a=AUG)[:, :, 0:DK],
            psv[:].rearrange("p (h a) -> p h a", a=DK))
        ones_col = vt[:].rearrange("p (h a) -> p h a", a=AUG)[:, :, DK]
        nc.vector.tensor_copy(ones_col, ones1[:].to_broadcast((128, H)))

    def _emit_half(cxc, jj, tt, oc, ysb, tail=False):
        t = 4 * jj + tt
        psy = ps_w.tile([128, 512], F32, tag="ps", name="ps")
        for p in range(PAIRS):
            nc.tensor.matmul(psy[:], cxc[p][:, ts(tt, 128)],
                             wo_sb[:, p * DM + oc * 512:
                                   p * DM + oc * 512 + 512],
                             start=(p == 0), stop=(p == PAIRS - 1))
        if tail and oc == 0:
            # scalar-engine copy so the two halves' copies run in parallel
            nc.scalar.copy(ysb[:, ts(oc, 512)], psy[:])
        else:
            nc.vector.tensor_copy(ysb[:, ts(oc, 512)], psy[:])
        nc.sync.dma_start(out=y[ts(t, 128), ts(oc, 512)],
                          in_=ysb[:, ts(oc, 512)])

    def _emit_tile(cxc, jj, tt, tail=False):
        ysb = yp.tile([128, DM], F16, tag="y", name="ysb")
        for oc in range(2):
            _emit_half(cxc, jj, tt, oc, ysb, tail=tail)

    # dummy matmuls on the zeroed warm tile keep the PE HAM activity
    # monitor busy while the first DMAs land, so chunk-0 runs at full clock
    for _ in range(18):
        pswm = ps_w.tile([128, 512], F32, tag="ps", name="ps")
        nc.tensor.matmul(pswm[:], warm[:, 0:128], warm[:],
                         start=True, stop=True)

    # chunk-0 projections run up front (nothing to overlap them with yet)
    qc_cur = []
    for p in range(PAIRS):
        _proj_pair(0, p, qc_cur)
    for tt in range(4):
        _vproj(0, tt)

    pending = None   # (jj, cxc) for the previous chunk's output projection
    carry = None     # (pa, pi, pc0, poa, pob, ha, hb, p, jd, cxl)
    txp = None       # (cxq, p, cxl) deferred transpose+copy

    def _attn_v(pa, pi, pc0, poa, pob, ha, hb, jd):
        # flipped attn@V: lhsT = exp-tile q-slice [128 keys, 128 q],
        # rhs = per-head V (+ones) [128 keys, 65] moving; out [128 q, 65]
        # per q-slice with the denominator in column 64.
        d0 = max(pc0 // 128, 0)
        for hs, po, h in ((0, poa, ha), (1, pob, hb)):
            for qs in range(d0, 4):
                # start only on the tensor's first matmul of the round:
                # start_tensor_calc pending-zeros the WHOLE tensor, so a
                # second start=True would wipe sibling regions' has_written
                nc.tensor.matmul(
                    po[:, qs * AUG:qs * AUG + AUG],
                    pa[:, hs * 512 + qs * 128:hs * 512 + qs * 128 + 128],
                    v_sb[pi][:, h * AUG:(h + 1) * AUG],
                    start=(pi == 0 and qs == d0), stop=(pi == 4 * jd + qs))

    def _drain(c, defer=False):
        # last key-tile's attn@V, then normalize + transpose. All po reads
        # happen here (before the next pair's first attn@V write reuses the
        # single-buffered oa/ob PSUM slots). Head a's DVE normalize runs
        # under head b's attn@V stream on the PE.
        pa_, pi_, pc0_, poa_, pob_, ha_, hb_, p_, jd, cxl = c
        d0 = max(pc0_ // 128, 0)
        cxq = cxp.tile([128, 512], F16, tag="cxq", name="cxq")

        def _norm(hs, po):
            # per-partition denominators: reciprocal of po[:, 64::65], one
            # broadcast multiply into the [q, 2h*64] staging tile
            r = rp.tile([128, 4], F32, tag="r", name="r")
            den = bass.AP(tensor=po.tensor, offset=po.offset + DK,
                          ap=[list(po.ap[0]), [AUG, 4]])
            nc.vector.reciprocal_approx_fast(r[:], den)
            po3 = bass.AP(tensor=po.tensor, offset=po.offset,
                          ap=[list(po.ap[0]), [AUG, 4], [1, DK]])
            cx3 = bass.AP(tensor=cxq.tensor, offset=cxq.offset + hs * DK,
                          ap=[list(cxq.ap[0]), [128, 4], [1, DK]])
            rb = bass.AP(tensor=r.tensor, offset=r.offset,
                         ap=[list(r.ap[0]), [1, 4], [0, DK]])
            nc.vector.tensor_mul(cx3, po3, rb)

        for qs in range(d0, 4):
            nc.tensor.matmul(
                poa_[:, qs * AUG:qs * AUG + AUG],
                pa_[:, qs * 128:qs * 128 + 128],
                v_sb[pi_][:, ha_ * AUG:(ha_ + 1) * AUG],
                start=(pi_ == 0 and qs == d0), stop=(pi_ == 4 * jd + qs))
        _norm(0, poa_)
        for qs in range(d0, 4):
            nc.tensor.matmul(
                pob_[:, qs * AUG:qs * AUG + AUG],
                pa_[:, 512 + qs * 128:512 + qs * 128 + 128],
                v_sb[pi_][:, hb_ * AUG:(hb_ + 1) * AUG],
                start=(pi_ == 0 and qs == d0), stop=(pi_ == 4 * jd + qs))
        _norm(1, pob_)
        if defer:
            # transposes deferred 2 iterations into the next pair so its
            # score matmuls hide the DVE normalize latency (the PE FIFO
            # would otherwise stall on cxq at the pair boundary)
            return (cxq, p_, cxl)
        _drain_fin(cxq, p_, cxl, in_oa=True)
        return None

    def _drain_fin(cxq, p_, cxl, in_oa):
        # PE transposes [q, dim] -> [dim, q] for the output projection.
        # Immediate fins ride the oa PSUM slot (fully read by now); deferred
        # fins use a ps_w slot instead (the next pair's attn@V reclaims oa
        # at iteration 1, before a deferred transpose would write it).
        if in_oa:
            tp = ps_o.tile([128, 512], F16, tag="oa", name="tp",
                           padded_shape=[128, 512])
        else:
            tp = ps_w.tile([128, 512], F16, tag="ps", name="ps")
        for qs in range(4):
            nc.tensor.transpose(tp[:, ts(qs, 128)], cxq[:, ts(qs, 128)],
                                idn_sb[:])
        cxT = cxp.tile([128, 512], F16, tag=f"cx{p_}", name=f"cx{p_}")
        nc.vector.tensor_copy(cxT[:], tp[:])
        cxl.append(cxT)

    for j in range(NQC):
        cx_list = []
        qc_next = []
        if 1 <= j and j + 1 < NQC:
            # boundary filler: V tiles 0-1 of chunk j+1 (inputs resident
            # since the prologue) keep the PE busy while the previous
            # chunk's final exps free the score-PSUM buffers
            _vproj(j + 1, 0)
            _vproj(j + 1, 1)

        for p in range(PAIRS):
            ha, hb = 2 * p, 2 * p + 1
            nk = 4 * j + 4
            poa = ps_o.tile([128, 4 * AUG], F32, tag="oa", name="oa",
                            padded_shape=[128, 512])
            pob = ps_o.tile([128, 4 * AUG], F32, tag="ob", name="ob",
                            padded_shape=[128, 512])
            # last chunk has no next-chunk projection to inject, so spread the
            # previous chunk's output-projection tile one matmul per key-tile
            fine_emit = (j + 1 == NQC and pending is not None)
            if fine_emit:
                jj0, cxc0 = pending
                ysb_cur = yp.tile([128, DM], F16, tag="y", name="ysb")
                psys = [ps_w.tile([128, 512], F32, tag="ps", name="ps")
                        for _ in range(2)]
                emit_ops = [('mm', oc, pp) for oc in range(2)
                            for pp in range(PAIRS)]
                emit_ops.insert(4, ('fin', 0, None))
                emit_ops.append(('fin', 1, None))
                # pair-start filler: the first three accumulation matmuls
                # depend only on long-ready cx pairs 0-2 of chunk j-1;
                # they bridge the pair-transition window where only the
                # previous pair's drain matmuls are otherwise available
                for _ in range(3):
                    kind, oc, pp = emit_ops.pop(0)
                    nc.tensor.matmul(
                        psys[oc][:], cxc0[pp][:, ts(p, 128)],
                        wo_sb[:, pp * DM + oc * 512:
                              pp * DM + oc * 512 + 512],
                        start=(pp == 0), stop=(pp == PAIRS - 1))
            prev = None
            for i in range(nk):
                d = i - 4 * j
                c0 = 128 * d if d > 0 else 0
                w = 512 - c0
                at = ap_.tile([128, 1024], F16, tag="at", name="at")
                sp = ps_s.tile([128, 1024], F32, tag="sp", name="sp")
                nc.tensor.matmul(sp[0:128, c0:512],
                                 kT[p][0:64, ts(i, 128)],
                                 qc_cur[p][0:64, bass.ds(c0, w)],
                                 start=True, stop=True)
                nc.tensor.matmul(sp[0:128, 512 + c0:1024],
                                 kT[p][64:128, ts(i, 128)],
                                 qc_cur[p][64:128, bass.ds(c0, w)],
                                 start=True, stop=True)
                if carry is not None:
                    _drain(carry)
                    carry = None
                if c0 == 0:
                    if d < 0 and i % 3 == 1:
                        # offload ~1/3 of full-tile exps to the DVE via the
                        # custom 8-stage quadratic-(1+u)^32 approximation
                        nc.vector._custom_dve(_EXPQ, out=at[:], in0=sp[:],
                                              s0=EC0, s1=0.5, imm2=0.0)
                    else:
                        nc.scalar.activation(at[:], sp[:],
                                             mybir.ActivationFunctionType.Exp,
                                             bias=ebias[:], scale=SQ2048)
                else:
                    sp_strip = bass.AP(tensor=sp.tensor, offset=sp.offset + c0,
                                       ap=[list(sp.ap[0]), [512, 2], [1, w]])
                    at_strip = bass.AP(tensor=at.tensor, offset=at.offset + c0,
                                       ap=[list(at.ap[0]), [512, 2], [1, w]])
                    nc.scalar.activation(at_strip, sp_strip,
                                         mybir.ActivationFunctionType.Exp,
                                         bias=ebias[:], scale=SQ2048)
                if d >= 0:
                    strip = bass.AP(tensor=at.tensor, offset=at.offset + c0,
                                    ap=[list(at.ap[0]), [512, 2], [1, 128]])
                    tri_b = bass.AP(tensor=tri_sb.tensor, offset=tri_sb.offset,
                                    ap=[list(tri_sb.ap[0]), [0, 2], [1, 128]])
                    nc.vector.tensor_mul(strip, strip, tri_b)
                if prev is not None:
                    pa, pi, pc0 = prev
                    _attn_v(pa, pi, pc0, poa, pob, ha, hb, j)
                if fine_emit and i >= 4 and emit_ops:
                    kind, oc, pp = emit_ops.pop(0)
                    if kind == 'mm':
                        nc.tensor.matmul(
                            psys[oc][:], cxc0[pp][:, ts(p, 128)],
                            wo_sb[:, pp * DM + oc * 512:
                                  pp * DM + oc * 512 + 512],
                            start=(pp == 0), stop=(pp == PAIRS - 1))
                    else:
                        nc.vector.tensor_copy(ysb_cur[:, ts(oc, 512)],
                                              psys[oc][:])
                        nc.sync.dma_start(
                            out=y[ts(4 * jj0 + p, 128), ts(oc, 512)],
                            in_=ysb_cur[:, ts(oc, 512)])
                prev = (at, i, c0)
            pa, pi, pc0 = prev
            carry = (pa, pi, pc0, poa, pob, ha, hb, p, j, cx_list)
            # interleave next-chunk projections and previous-chunk output
            # projection into the ACT-paced attention stretch
            if j + 1 < NQC:
                _proj_pair(j + 1, p, qc_next)
                if j == 0:
                    _vproj(j + 1, p)
                elif p < PAIRS - 2:
                    _vproj(j + 1, p + 2)
            if pending is not None and not fine_emit:
                _emit_tile(pending[1], pending[0], p)
        _drain(carry)
        carry = None

        pending = (j, cx_list)
        qc_cur = qc_next
        if j == NQC - 1:
            for tt in range(4):
                _emit_tile(cx_list, j, tt, tail=True)

_NC_CACHE = None


def _build():
    global _NC_CACHE
    if _NC_CACHE is None:
        from contextlib import ExitStack
        nc = bacc.Bacc("TRN2", target_bir_lowering=False, debug=False,
                       num_devices=N_CORES)
        with tile.TileContext(nc) as tc:
            with ExitStack() as ctx:
                _kernel_body(ctx, tc)
        nc.compile()
        _NC_CACHE = nc
    return _NC_CACHE


def _make_tri():
    K = np.arange(128)[:, None]
    Q = np.arange(128)[None, :]
    return (Q >= K).astype(np.float16)


def kernel(x, W_q, W_k, W_v, W_o, _trace=False, _tmpdir=None):
    x = np.asarray(x, dtype=np.float32)
    tri = _make_tri()
    idn = np.eye(128, dtype=np.float16)
    f16 = np.float16

    def _wblk_i(W, rows):
        # i-major: [128, 8*512] with contraction block i at cols i*512..
        wT = np.ascontiguousarray(np.asarray(W)[rows, :].T)  # [1024, 512]
        return np.ascontiguousarray(
            wT.reshape(NKT, 128, 512).transpose(1, 0, 2).reshape(128, NKT * 512)
        ).astype(f16)

    def _wblk_p(W, rows):
        # pair-major: [128, p*1024 + i*128 + c]
        wT = np.ascontiguousarray(np.asarray(W)[rows, :].T)  # [1024, 512]
        return np.ascontiguousarray(
            wT.reshape(NKT, 128, PAIRS, 128).transpose(1, 2, 0, 3).reshape(
                128, PAIRS * 1024)).astype(f16)

    in_maps = []
    for c in range(N_CORES):
        b, g = divmod(c, 2)
        rows = slice(512 * g, 512 * (g + 1))
        xT = np.ascontiguousarray(x[b].T)  # [1024, 2048]
        xtr = np.ascontiguousarray(
            xT.reshape(NKT, 128, S).transpose(1, 0, 2)).astype(f16)
        woT = np.ascontiguousarray(np.asarray(W_o)[:, rows].T)  # [512, 1024]
        wor = np.ascontiguousarray(
            woT.reshape(PAIRS, 128, DM).transpose(1, 0, 2).reshape(
                128, PAIRS * DM)).astype(f16)
        in_maps.append({
            "xtr": xtr,
            "wqr": _wblk_p(W_q, rows),
            "wkr": _wblk_p(W_k, rows),
            "wvr": _wblk_i(W_v, rows),
            "wor": wor,
            "tri": tri,
            "idn": idn,
        })
    nc = _build()
    res = run_bass_kernel_spmd(nc, in_maps, core_ids=list(range(N_CORES)),
                               trace=_trace, tmpdir=_tmpdir)
    out = np.stack([res.results[2 * b]["y"].astype(np.float32)
                    + res.results[2 * b + 1]["y"].astype(np.float32)
                    for b in range(B)])
    kernel._last_exec_time_ns = res.exec_time_ns
    kernel._last_results = res
    return out
